# revision 18
# baseline (speedup 1.0000x reference)
"""Expert-parallel MoE GEGLU MLP (RMSNorm -> c_fc -> GEGLU -> c_proj) on 8
Trainium2 NeuronCores.

Sharding: expert-parallel. Core e computes the full MLP for expert e's tokens
(x[:, e] -> [8192, 768]); no collectives. gamma*sqrt(D) is folded into c_fc
and mult_bias into c_proj on the host.

The RMSNorm scale is DEFERRED past GEMM1. GEMM1 consumes the raw transposed
activations straight from DRAM, the per-token rsqrt scale is applied to the
GATE half right before gelu (tokens ride the free axis there, via a
partition-broadcast sc buffer built by gpsimd), and the VALUE half's scale is
folded into the GEMM2 output copy, where tokens sit on PSUM partitions, as
the ACT engine's per-partition scale:

    u_v = x @ W1_v ; u_g = x @ W1_g          (bf16 x bf16 -> fp32 PSUM)
    g   = gelu(u_g * s_tok) * u_v            (broadcast s on gate only)
    out = s_tok * (g @ W2)                   (per-partition scale on ACT)

This unblocks the pipeline head: the first GEMM1 chain only needs the first
w1 column block and half an xt super-block. All bulk loads are single-
doorbell 3-level-AP DMAs (the per-chunk variant was doorbell-issue-bound at
~0.65us per DMA_DIRECT2D): w1 value/gate blocks + w2 stream on the sync HW
queue, xt on scalar, xb on gpsimd (only sync/scalar/gpsimd can issue DMAs).
GEMM2 uses the GEGLU output chunks as the stationary operand so its PSUM
output is token-major; outputs DMA out per 512/256-column half, alternating
the sync and scalar HW queues.
"""

from contextlib import ExitStack

import ml_dtypes
import numpy as np

import concourse.bass as bass
import concourse.mybir as mybir
import concourse.tile as tile
from concourse import bacc
from concourse.bass_utils import run_bass_kernel_spmd
from concourse.masks import make_identity

# Problem dims (fixed by the nn_MLP_90795608637901 spec).
B, E, CAP, D = 8, 8, 1024, 768
H = 2048
H2 = 2 * H
T = B * CAP          # tokens per expert (per core) = 8192
SB = 1024            # tokens per super-block
NSB = T // SB        # 8
S = SB // 128        # 8 partition sub-tiles per super-block
KC1 = D // 128       # 6 contraction chunks for GEMM1
MC = H // 128        # 16 value/gate chunk pairs
KC2 = H // 128       # 16 contraction chunks for GEMM2

BF = mybir.dt.bfloat16
F32 = mybir.dt.float32
I32 = mybir.dt.int32
ALU = mybir.AluOpType
AF = mybir.ActivationFunctionType

# gpsimd partition_broadcast for the sc buffer; falls back to bf16 selector
# matmuls on the PE when disabled.
USE_PBCAST = True


def build_kernel(nsb: int = NSB) -> bass.Bass:
    nc = bacc.Bacc("TRN2", target_bir_lowering=False, debug=False)

    t = nsb * SB
    x = nc.declare_dram_parameter("x", [t, D], BF, isOutput=False)
    xT = nc.declare_dram_parameter("xT", [D, t], BF, isOutput=False)
    w1 = nc.declare_dram_parameter("w1", [D, H2], BF, isOutput=False)
    w2 = nc.declare_dram_parameter("w2", [H, D], BF, isOutput=False)
    out = nc.declare_dram_parameter("out", [t, D], BF, isOutput=True)

    xTv = xT.rearrange("(k p) t -> p k t", p=128)    # [128, KC1, t]
    w1v = w1.rearrange("(k p) h -> p k h", p=128)    # [128, KC1, H2]
    w2v = w2.rearrange("(k p) d -> p k d", p=128)    # [128, KC2, D]

    with tile.TileContext(nc) as tc, ExitStack() as ctx:
        weights = ctx.enter_context(tc.tile_pool(name="weights", bufs=1))
        io_in = ctx.enter_context(tc.tile_pool(name="io_in", bufs=2))
        work = ctx.enter_context(tc.tile_pool(name="work", bufs=2))
        gpool = ctx.enter_context(tc.tile_pool(name="gpool", bufs=1))
        small = ctx.enter_context(tc.tile_pool(name="small", bufs=2))
        scp = ctx.enter_context(tc.tile_pool(name="scp", bufs=2))
        agp = ctx.enter_context(tc.tile_pool(name="agp", bufs=3))
        pgp = ctx.enter_context(tc.tile_pool(name="pgp", bufs=3))
        obp = ctx.enter_context(tc.tile_pool(name="obp", bufs=3))
        n_mm = 7 if USE_PBCAST else 6
        psum_mm = ctx.enter_context(
            tc.tile_pool(name="psum_mm", bufs=n_mm, space="PSUM"))
        if not USE_PBCAST:
            psum_sc = ctx.enter_context(
                tc.tile_pool(name="psum_sc", bufs=1, space="PSUM"))
        psum_yt = ctx.enter_context(tc.tile_pool(name="psum_yt", bufs=1, space="PSUM"))

        x_tiles = {}

        def issue_x(sb, startup=False):
            # xt: d-major (host-transposed), feeds GEMM1 raw. One doorbell
            # per column half; the h2=0 sweep starts on half a super-block.
            xt = work.tile([128, KC1, SB], BF, name="xt", tag="xt")
            for h in range(2):
                eng = nc.gpsimd if (startup and h == 1) else nc.scalar
                eng.dma_start(
                    out=xt[:, :, h * 512:(h + 1) * 512],
                    in_=xTv[:, :, sb * SB + h * 512:sb * SB + (h + 1) * 512],
                )
            # xb: token-major, feeds the squared-sum only; two half DMAs so
            # the ACT pass can start on the first half. sb0's halves ride the
            # scalar HW queue behind xt0h0 (the ring is too slow for the
            # sc(0) deadline); steady-state halves go to the gpsimd ring.
            xb = io_in.tile([128, S, D], BF, name="xb", tag="xb")
            for h in range(2):
                xv = x[sb * SB + h * 512:sb * SB + (h + 1) * 512]
                eng = nc.scalar if startup else nc.gpsimd
                eng.dma_start(
                    out=xb[:, h * 4:(h + 1) * 4, :],
                    in_=xv.rearrange("(s p) d -> p s d", p=128),
                )
            x_tiles[sb] = (xb, xt)

        ident = weights.tile([128, 128], F32)
        make_identity(nc, ident)
        if not USE_PBCAST:
            sels = weights.tile([S, SB], BF)
            seltmp = weights.tile([S, SB], F32)
            nc.gpsimd.memset(seltmp, 0.0)
            for s in range(S):
                nc.gpsimd.memset(seltmp[s:s + 1, s * 128:(s + 1) * 128], 1.0)
            nc.vector.tensor_copy(sels, seltmp)
        bias0 = weights.tile([128, 1], F32)
        nc.vector.memset(bias0, 0.0)

        # Startup-ordered head. The HW DGE processes ~180 descriptors/us, so
        # a [128,KC1,cols] block costs ~4.3us queue time regardless of cols —
        # never split below 512 columns. Value/gate blocks interleave on the
        # sync queue in consumption order; w2 takes the gpsimd ring.
        w1s = weights.tile([128, KC1, H2], BF)
        w2s = weights.tile([128, KC2, D], BF)
        issue_x(0, startup=True)

        # Value/gate blocks interleaved on the sync queue in consumption
        # order; the HW DGE runs ~180 descriptors/us, so a [128,KC1,cols]
        # block costs ~4.3us regardless of cols — never split below 512.
        for nb in range(4):
            c0, c1 = nb * 512, (nb + 1) * 512
            nc.sync.dma_start(out=w1s[:, :, c0:c1], in_=w1v[:, :, c0:c1])
            nc.sync.dma_start(out=w1s[:, :, H + c0:H + c1],
                              in_=w1v[:, :, H + c0:H + c1])

        scales = {}

        def scale_pipeline(sb):
            xb, _ = x_tiles[sb]
            # --- per-token squared sum, token-major: ss on ACT ---
            ssb = small.tile([128, S], F32, name="ssb")
            sq = small.tile([128, D], BF, name="sq")
            for s in range(S):
                nc.scalar.activation(
                    sq, xb[:, s], AF.Square,
                    bias=bias0, accum_out=ssb[:, s:s + 1],
                )
            yb = small.tile([128, S], F32, name="yb")
            tb = small.tile([128, S], F32, name="tb")
            # rsqrt seed via the int bit trick: 0x5f3759df - (i >> 1)
            # (written as (i>>1 xor -1) + 0x5f3759df + 1), then 3 Newton steps.
            nc.vector.tensor_scalar(
                out=yb.bitcast(I32), in0=ssb.bitcast(I32),
                scalar1=1, scalar2=-1,
                op0=ALU.logical_shift_right, op1=ALU.bitwise_xor,
            )
            nc.vector.tensor_scalar(
                out=yb.bitcast(I32), in0=yb.bitcast(I32),
                scalar1=0x5F375A60, scalar2=None, op0=ALU.add,
            )
            for _ in range(3):
                nc.vector.tensor_mul(tb, yb, yb)
                nc.vector.tensor_mul(tb, tb, ssb)
                nc.vector.tensor_scalar(
                    out=tb, in0=tb, scalar1=-0.5, scalar2=1.5,
                    op0=ALU.mult, op1=ALU.add,
                )
                nc.vector.tensor_mul(yb, yb, tb)

            # --- broadcast scale across partitions: yb[p,s] -> sc[:,s*128+p]
            # via one tiny PE transpose, a partition-0 gather DMA (on the
            # scalar HW queue — the ring FIFO would sit behind bulk loads),
            # and one gpsimd partition_broadcast ---
            yt = psum_yt.tile([S, 128], F32, name="yt", tag="yt", space="PSUM")
            nc.tensor.transpose(yt, yb, ident)
            sc = scp.tile([128, SB], F32, name="sc", tag="sc")
            yrow = small.tile([S, 128], F32, name="yrow")
            nc.vector.tensor_copy(yrow, yt)
            yrow1 = small.tile([1, SB], F32, name="yrow1")
            nc.gpsimd.dma_start(out=yrow1, in_=yrow)
            nc.gpsimd.partition_broadcast(sc, yrow1)
            scales[sb] = (yb, sc)

        scale_pipeline(0)
        # w2 rides the gpsimd ring, emitted after the sb0 scale pipeline so
        # its yrow gather DMA isn't stuck behind 3MB in the ring FIFO; w2 is
        # only needed when GEMM2(sb0) starts ~90us in.
        for k0 in (0, 8):
            nc.gpsimd.dma_start(out=w2s[:, k0:k0 + 8, :], in_=w2v[:, k0:k0 + 8, :])
        for sb in range(nsb):
            if sb + 1 < nsb:
                issue_x(sb + 1)
            _, xt = x_tiles.pop(sb)
            yb, sc = scales.pop(sb)

            # --- GEMM1 + GEGLU on raw xt, one value/gate chunk pair at a
            # time; h2 outer so the sweep only needs half an xt block. A
            # matmul's fp32 PSUM output cannot cross a 2KB bank, so the
            # 1024-token super-block runs as two 512-column halves. ---
            gbuf = gpool.tile([128, KC2, SB], BF, name="gbuf")
            for h2 in range(2):
                cols = slice(h2 * 512, (h2 + 1) * 512)

                def chain(m, base, name):
                    p = psum_mm.tile([128, 512], F32, name=name, tag="mm",
                                     space="PSUM")
                    for k in range(KC1):
                        nc.tensor.matmul(
                            p, lhsT=w1s[:, k, base + m * 128:base + (m + 1) * 128],
                            rhs=xt[:, k, cols],
                            start=(k == 0), stop=(k == KC1 - 1),
                        )
                    return p

                # Value chains run one ahead of gate chains (v0 v1 g0 v2 g1
                # ...) so the gate weight stream has an extra chain of slack.
                pv_t = {0: chain(0, 0, "pv"), 1: chain(1, 0, "pv")}
                for m in range(MC):
                    pg = chain(m, H, "pg")
                    if m + 2 < MC:
                        pv_t[m + 2] = chain(m + 2, 0, "pv")
                    pv = pv_t.pop(m)
                    pgs = pgp.tile([128, 512], F32, name="pgs")
                    nc.vector.tensor_mul(pgs, pg, sc[:, cols])
                    ag = agp.tile([128, 512], F32, name="ag")
                    nc.scalar.activation(ag, pgs, AF.Gelu, bias=bias0)
                    nc.vector.tensor_mul(gbuf[:, m, cols], pv, ag)

            if sb + 1 < nsb:
                scale_pipeline(sb + 1)

            # --- GEMM2 with gbuf chunks stationary: PSUM comes out
            # token-major, so the deferred per-token scale rides the ACT
            # copy's per-partition scale operand and results DMA straight
            # out. d=768 splits into 512+256 PSUM chains (bank rule). ---
            for mt in range(S):
                ob = obp.tile([128, D], BF, name="ob")
                for d0, d1 in ((0, 512), (512, 768)):
                    po = psum_mm.tile([128, d1 - d0], F32, name="po", tag="mm",
                                      space="PSUM")
                    for k2 in range(KC2):
                        nc.tensor.matmul(
                            po, lhsT=gbuf[:, k2, mt * 128:(mt + 1) * 128],
                            rhs=w2s[:, k2, d0:d1],
                            start=(k2 == 0), stop=(k2 == KC2 - 1),
                        )
                    nc.scalar.activation(
                        ob[:, d0:d1], po, AF.Copy, bias=0.0,
                        scale=yb[:, mt:mt + 1],
                    )
                    eng = nc.sync if d0 == 0 else nc.scalar
                    eng.dma_start(
                        out=out[sb * SB + mt * 128:sb * SB + (mt + 1) * 128,
                                d0:d1],
                        in_=ob[:, d0:d1],
                    )

    nc.finalize()
    return nc


def prepare_in_maps(x, c_fc, c_proj, gamma, mult_bias):
    bf16 = ml_dtypes.bfloat16
    g = (gamma.astype(np.float32) * np.float32(np.sqrt(D)))
    w1_all = (c_fc.astype(np.float32) * g[None, :, None]).astype(bf16)
    w2_all = (c_proj.astype(np.float32)
              * mult_bias.astype(np.float32)[None, :, None]).astype(bf16)
    xs = np.ascontiguousarray(np.transpose(x, (1, 0, 2, 3))).reshape(E, T, D)
    xs = xs.astype(bf16)
    xts = np.ascontiguousarray(np.transpose(xs, (0, 2, 1)))
    return [
        {"x": xs[e], "xT": xts[e], "w1": w1_all[e], "w2": w2_all[e]}
        for e in range(E)
    ]


def run(in_maps, trace: bool = False):
    nc = build_kernel()
    return run_bass_kernel_spmd(
        nc, in_maps, core_ids=list(range(E)), trace=trace,
    )


def kernel(x, c_fc, c_proj, gamma, mult_bias):
    in_maps = prepare_in_maps(x, c_fc, c_proj, gamma, mult_bias)
    res = run(in_maps)
    out = np.empty((E, B, CAP, D), np.float32)
    for e in range(E):
        out[e] = res.results[e]["out"].astype(np.float32).reshape(B, CAP, D)
    return np.ascontiguousarray(out.transpose(1, 0, 2, 3))


# revision 21
# speedup vs baseline: 1.0118x; 1.0118x over previous
"""Expert-parallel MoE GEGLU MLP (RMSNorm -> c_fc -> GEGLU -> c_proj) on 8
Trainium2 NeuronCores.

Sharding: expert-parallel. Core e computes the full MLP for expert e's tokens
(x[:, e] -> [8192, 768]); no collectives. gamma*sqrt(D) is folded into c_fc
and mult_bias into c_proj on the host.

The RMSNorm scale is DEFERRED past GEMM1. GEMM1 consumes the raw transposed
activations straight from DRAM, the per-token rsqrt scale is applied to the
GATE half right before gelu (tokens ride the free axis there, via a
partition-broadcast sc buffer built by gpsimd), and the VALUE half's scale is
folded into the GEMM2 output copy, where tokens sit on PSUM partitions, as
the ACT engine's per-partition scale:

    u_v = x @ W1_v ; u_g = x @ W1_g          (bf16 x bf16 -> fp32 PSUM)
    g   = gelu(u_g * s_tok) * u_v            (broadcast s on gate only)
    out = s_tok * (g @ W2)                   (per-partition scale on ACT)

This unblocks the pipeline head: the first GEMM1 chain only needs the first
w1 column block and half an xt super-block. All bulk loads are single-
doorbell 3-level-AP DMAs (the per-chunk variant was doorbell-issue-bound at
~0.65us per DMA_DIRECT2D): w1 value/gate blocks + w2 stream on the sync HW
queue, xt on scalar, xb on gpsimd (only sync/scalar/gpsimd can issue DMAs).
GEMM2 uses the GEGLU output chunks as the stationary operand so its PSUM
output is token-major; outputs DMA out per 512/256-column half, alternating
the sync and scalar HW queues.
"""

from contextlib import ExitStack

import ml_dtypes
import numpy as np

import concourse.bass as bass
import concourse.mybir as mybir
import concourse.tile as tile
from concourse import bacc
from concourse.bass_utils import run_bass_kernel_spmd
from concourse.masks import make_identity

# Problem dims (fixed by the nn_MLP_90795608637901 spec).
B, E, CAP, D = 8, 8, 1024, 768
H = 2048
H2 = 2 * H
T = B * CAP          # tokens per expert (per core) = 8192
SB = 1024            # tokens per super-block
NSB = T // SB        # 8
S = SB // 128        # 8 partition sub-tiles per super-block
KC1 = D // 128       # 6 contraction chunks for GEMM1
MC = H // 128        # 16 value/gate chunk pairs
KC2 = H // 128       # 16 contraction chunks for GEMM2

BF = mybir.dt.bfloat16
F32 = mybir.dt.float32
I32 = mybir.dt.int32
ALU = mybir.AluOpType
AF = mybir.ActivationFunctionType

# gpsimd partition_broadcast for the sc buffer; falls back to bf16 selector
# matmuls on the PE when disabled.
USE_PBCAST = True


def build_kernel(nsb: int = NSB) -> bass.Bass:
    nc = bacc.Bacc("TRN2", target_bir_lowering=False, debug=False)

    t = nsb * SB
    x = nc.declare_dram_parameter("x", [t, D], BF, isOutput=False)
    xT = nc.declare_dram_parameter("xT", [D, t], BF, isOutput=False)
    w1 = nc.declare_dram_parameter("w1", [D, H2], BF, isOutput=False)
    w2 = nc.declare_dram_parameter("w2", [H, D], BF, isOutput=False)
    out = nc.declare_dram_parameter("out", [t, D], BF, isOutput=True)

    xTv = xT.rearrange("(k p) t -> p k t", p=128)    # [128, KC1, t]
    w1v = w1.rearrange("(k p) h -> p k h", p=128)    # [128, KC1, H2]
    w2v = w2.rearrange("(k p) d -> p k d", p=128)    # [128, KC2, D]

    with tile.TileContext(nc) as tc, ExitStack() as ctx:
        weights = ctx.enter_context(tc.tile_pool(name="weights", bufs=1))
        io_in = ctx.enter_context(tc.tile_pool(name="io_in", bufs=2))
        work = ctx.enter_context(tc.tile_pool(name="work", bufs=2))
        gpool = ctx.enter_context(tc.tile_pool(name="gpool", bufs=1))
        small = ctx.enter_context(tc.tile_pool(name="small", bufs=2))
        scp = ctx.enter_context(tc.tile_pool(name="scp", bufs=2))
        agp = ctx.enter_context(tc.tile_pool(name="agp", bufs=3))
        pgp = ctx.enter_context(tc.tile_pool(name="pgp", bufs=3))
        obp = ctx.enter_context(tc.tile_pool(name="obp", bufs=3))
        n_mm = 7 if USE_PBCAST else 6
        psum_mm = ctx.enter_context(
            tc.tile_pool(name="psum_mm", bufs=n_mm, space="PSUM"))
        if not USE_PBCAST:
            psum_sc = ctx.enter_context(
                tc.tile_pool(name="psum_sc", bufs=1, space="PSUM"))
        psum_yt = ctx.enter_context(tc.tile_pool(name="psum_yt", bufs=1, space="PSUM"))

        x_tiles = {}

        def issue_x(sb, startup=False):
            # xb: token-major, feeds the squared-sum only; two half DMAs on
            # gpsimd so the ACT pass can start on the first half.
            xb = io_in.tile([128, S, D], BF, name="xb", tag="xb")
            for h in range(2):
                xv = x[sb * SB + h * 512:sb * SB + (h + 1) * 512]
                nc.gpsimd.dma_start(
                    out=xb[:, h * 4:(h + 1) * 4, :],
                    in_=xv.rearrange("(s p) d -> p s d", p=128),
                )
            # xt: d-major (host-transposed), feeds GEMM1 raw. One doorbell
            # per column half; the h2=0 sweep starts on half a super-block.
            # Both halves ride the scalar HW queue — keeping xt0h1 off the
            # gpsimd ring lets xb0 (and then the sc(0) yrow gather) through
            # ~10us earlier, and the h2=1 sweep only needs h1 ~40us in.
            xt = work.tile([128, KC1, SB], BF, name="xt", tag="xt")
            for h in range(2):
                nc.scalar.dma_start(
                    out=xt[:, :, h * 512:(h + 1) * 512],
                    in_=xTv[:, :, sb * SB + h * 512:sb * SB + (h + 1) * 512],
                )
            x_tiles[sb] = (xb, xt)

        ident = weights.tile([128, 128], F32)
        make_identity(nc, ident)
        if not USE_PBCAST:
            sels = weights.tile([S, SB], BF)
            seltmp = weights.tile([S, SB], F32)
            nc.gpsimd.memset(seltmp, 0.0)
            for s in range(S):
                nc.gpsimd.memset(seltmp[s:s + 1, s * 128:(s + 1) * 128], 1.0)
            nc.vector.tensor_copy(sels, seltmp)
        bias0 = weights.tile([128, 1], F32)
        nc.vector.memset(bias0, 0.0)

        # Startup-ordered head: first value/gate column blocks split at 128
        # columns, then 512-column value/gate pairs, then w2 — all single
        # 3-level-AP doorbells on the sync HW queue, in the order the first
        # GEMM1 chains consume them.
        w1s = weights.tile([128, KC1, H2], BF)
        w2s = weights.tile([128, KC2, D], BF)
        issue_x(0, startup=True)

        def w1_block(c0, c1):
            nc.sync.dma_start(out=w1s[:, :, c0:c1], in_=w1v[:, :, c0:c1])

        w1_block(0, 128)
        w1_block(128, 512)
        w1_block(H, H + 128)
        w1_block(H + 128, H + 512)
        for nb in range(1, 4):
            w1_block(nb * 512, (nb + 1) * 512)
            w1_block(H + nb * 512, H + (nb + 1) * 512)
        for k0 in (0, 8):
            nc.sync.dma_start(out=w2s[:, k0:k0 + 8, :], in_=w2v[:, k0:k0 + 8, :])

        scales = {}

        def scale_pipeline(sb):
            xb, _ = x_tiles[sb]
            # --- per-token squared sum, token-major: ss on ACT ---
            ssb = small.tile([128, S], F32, name="ssb")
            sq = small.tile([128, D], BF, name="sq")
            for s in range(S):
                nc.scalar.activation(
                    sq, xb[:, s], AF.Square,
                    bias=bias0, accum_out=ssb[:, s:s + 1],
                )
            yb = small.tile([128, S], F32, name="yb")
            tb = small.tile([128, S], F32, name="tb")
            # rsqrt seed via the int bit trick: 0x5f3759df - (i >> 1)
            # (written as (i>>1 xor -1) + 0x5f3759df + 1), then 3 Newton steps.
            nc.vector.tensor_scalar(
                out=yb.bitcast(I32), in0=ssb.bitcast(I32),
                scalar1=1, scalar2=-1,
                op0=ALU.logical_shift_right, op1=ALU.bitwise_xor,
            )
            nc.vector.tensor_scalar(
                out=yb.bitcast(I32), in0=yb.bitcast(I32),
                scalar1=0x5F375A60, scalar2=None, op0=ALU.add,
            )
            for _ in range(3):
                nc.vector.tensor_mul(tb, yb, yb)
                nc.vector.tensor_mul(tb, tb, ssb)
                nc.vector.tensor_scalar(
                    out=tb, in0=tb, scalar1=-0.5, scalar2=1.5,
                    op0=ALU.mult, op1=ALU.add,
                )
                nc.vector.tensor_mul(yb, yb, tb)

            # --- broadcast scale across partitions: yb[p,s] -> sc[:,s*128+p]
            # via one tiny PE transpose, a partition-0 gather DMA (on the
            # scalar HW queue — the ring FIFO would sit behind bulk loads),
            # and one gpsimd partition_broadcast ---
            yt = psum_yt.tile([S, 128], F32, name="yt", tag="yt", space="PSUM")
            nc.tensor.transpose(yt, yb, ident)
            sc = scp.tile([128, SB], F32, name="sc", tag="sc")
            yrow = small.tile([S, 128], F32, name="yrow")
            nc.vector.tensor_copy(yrow, yt)
            yrow1 = small.tile([1, SB], F32, name="yrow1")
            nc.gpsimd.dma_start(out=yrow1, in_=yrow)
            nc.gpsimd.partition_broadcast(sc, yrow1)
            scales[sb] = (yb, sc)

        scale_pipeline(0)
        for sb in range(nsb):
            if sb + 1 < nsb:
                issue_x(sb + 1)
            _, xt = x_tiles.pop(sb)
            yb, sc = scales.pop(sb)

            # --- GEMM1 + GEGLU on raw xt, one value/gate chunk pair at a
            # time; h2 outer so the sweep only needs half an xt block. A
            # matmul's fp32 PSUM output cannot cross a 2KB bank, so the
            # 1024-token super-block runs as two 512-column halves. ---
            gbuf = gpool.tile([128, KC2, SB], BF, name="gbuf")
            for h2 in range(2):
                cols = slice(h2 * 512, (h2 + 1) * 512)

                def chain(m, base, name):
                    p = psum_mm.tile([128, 512], F32, name=name, tag="mm",
                                     space="PSUM")
                    for k in range(KC1):
                        nc.tensor.matmul(
                            p, lhsT=w1s[:, k, base + m * 128:base + (m + 1) * 128],
                            rhs=xt[:, k, cols],
                            start=(k == 0), stop=(k == KC1 - 1),
                        )
                    return p

                # Value chains run one ahead of gate chains (v0 v1 g0 v2 g1
                # ...) so the gate weight stream has an extra chain of slack.
                pv_t = {0: chain(0, 0, "pv"), 1: chain(1, 0, "pv")}
                for m in range(MC):
                    pg = chain(m, H, "pg")
                    if m + 2 < MC:
                        pv_t[m + 2] = chain(m + 2, 0, "pv")
                    pv = pv_t.pop(m)
                    pgs = pgp.tile([128, 512], F32, name="pgs")
                    nc.vector.tensor_mul(pgs, pg, sc[:, cols])
                    ag = agp.tile([128, 512], F32, name="ag")
                    nc.scalar.activation(ag, pgs, AF.Gelu, bias=bias0)
                    nc.vector.tensor_mul(gbuf[:, m, cols], pv, ag)

            if sb + 1 < nsb:
                scale_pipeline(sb + 1)

            # --- GEMM2 with gbuf chunks stationary: PSUM comes out
            # token-major, so the deferred per-token scale rides the ACT
            # copy's per-partition scale operand and results DMA straight
            # out. d=768 splits into 512+256 PSUM chains (bank rule). ---
            for mt in range(S):
                ob = obp.tile([128, D], BF, name="ob")
                for d0, d1 in ((0, 512), (512, 768)):
                    po = psum_mm.tile([128, d1 - d0], F32, name="po", tag="mm",
                                      space="PSUM")
                    for k2 in range(KC2):
                        nc.tensor.matmul(
                            po, lhsT=gbuf[:, k2, mt * 128:(mt + 1) * 128],
                            rhs=w2s[:, k2, d0:d1],
                            start=(k2 == 0), stop=(k2 == KC2 - 1),
                        )
                    nc.scalar.activation(
                        ob[:, d0:d1], po, AF.Copy, bias=0.0,
                        scale=yb[:, mt:mt + 1],
                    )
                    eng = nc.sync if d0 == 0 else nc.scalar
                    eng.dma_start(
                        out=out[sb * SB + mt * 128:sb * SB + (mt + 1) * 128,
                                d0:d1],
                        in_=ob[:, d0:d1],
                    )

    nc.finalize()
    return nc


def prepare_in_maps(x, c_fc, c_proj, gamma, mult_bias):
    bf16 = ml_dtypes.bfloat16
    g = (gamma.astype(np.float32) * np.float32(np.sqrt(D)))
    w1_all = (c_fc.astype(np.float32) * g[None, :, None]).astype(bf16)
    w2_all = (c_proj.astype(np.float32)
              * mult_bias.astype(np.float32)[None, :, None]).astype(bf16)
    xs = np.ascontiguousarray(np.transpose(x, (1, 0, 2, 3))).reshape(E, T, D)
    xs = xs.astype(bf16)
    xts = np.ascontiguousarray(np.transpose(xs, (0, 2, 1)))
    return [
        {"x": xs[e], "xT": xts[e], "w1": w1_all[e], "w2": w2_all[e]}
        for e in range(E)
    ]


def run(in_maps, trace: bool = False):
    nc = build_kernel()
    return run_bass_kernel_spmd(
        nc, in_maps, core_ids=list(range(E)), trace=trace,
    )


def kernel(x, c_fc, c_proj, gamma, mult_bias):
    in_maps = prepare_in_maps(x, c_fc, c_proj, gamma, mult_bias)
    res = run(in_maps)
    out = np.empty((E, B, CAP, D), np.float32)
    for e in range(E):
        out[e] = res.results[e]["out"].astype(np.float32).reshape(B, CAP, D)
    return np.ascontiguousarray(out.transpose(1, 0, 2, 3))
